# revision 67
# baseline (speedup 1.0000x reference)
"""Fused LayerNorm + 16-head self-attention + output projection on 8 NeuronCores.

Sharding: core c = (batch b = c//2, head-group g = c%2).  Data parallel over
the 4 batches; tensor parallel over head groups (8 heads each, Megatron-style
column split of W_q/W_kv and row split of W_out).  The two partial outputs
per batch are summed on the host.

v3 design notes (single-pass pipelined schedule, all-bf16 matmuls):
  The scalar engine's softmax exp stream (256 x [128,1024] ~= 285us) and the
  PE matmul stream (~420us of slices) are co-bottlenecks; the schedule keeps
  both streaming with no stage barriers.
  - q is processed in four 512-wide blocks (qb); heads in four pairs (p).
    Per (p,qb,kc): two score matmuls land in one [128,1024] PSUM tile
    (the two heads run concurrently on disjoint PE row groups 0-63/64-127),
    ONE exp covers both heads, two AV matmuls accumulate per-head
    [65,512] outputs (ones column in V gives the softmax denominator).
    The kc loop is software-pipelined: scores(kc+1) issue before AV(kc)
    so the PE never head-of-line blocks on the exp it feeds.
  - Stage-B PSUM = scores 2x2 banks + AV 2x1 = 6 banks; a 2-bank filler
    pool serves everything else (late LN/transpose/v tiles, remaining
    q/k projections, out-proj) interleaved between attention groups as
    PE gap-filler under the ACT exp stream.
  - LN apply runs on GpSimd (tensor_scalar with per-partition -mu/rstd),
    keeping ACT exp-only during steady state.
"""

from contextlib import ExitStack

import numpy as np
import ml_dtypes

import concourse.bacc as bacc
import concourse.tile as tile
from concourse import mybir
from concourse.bass_utils import run_bass_kernel_spmd
from concourse.masks import make_identity

F32 = mybir.dt.float32
BF16 = mybir.dt.bfloat16

B, N, D = 4, 2048, 1024
H_TOT, DH, E = 16, 64, 1024
NCORES = 8
HL = 8            # heads per core
EL = HL * DH      # 512 local embed
NT = N // 128     # 16 token tiles
NDC = D // 128    # 8 contraction chunks
NP = 4            # head pairs per core
QB = 4            # q blocks of 512
SCALE = float(DH) ** -0.5
EPS = 1e-5

_nc_cache = {}


class _Kern:
    def __init__(self, dump=False):
        self.dump = dump
        self.nc = bacc.Bacc("TRN2", target_bir_lowering=False)
        nc = self.nc
        self.x = nc.dram_tensor("x", [N, D], BF16, kind="ExternalInput").ap()
        self.wq = nc.dram_tensor("wq", [D, EL], BF16, kind="ExternalInput").ap()
        self.wk = nc.dram_tensor("wk", [D, EL], BF16, kind="ExternalInput").ap()
        self.wv = nc.dram_tensor("wv", [D, EL], BF16, kind="ExternalInput").ap()
        self.wo = nc.dram_tensor("wo", [EL, D], BF16, kind="ExternalInput").ap()
        self.out = nc.dram_tensor("out", [N, D], F32, kind="ExternalOutput").ap()
        self.wqk_sb = {}
        if dump:
            self.d_xnt = nc.dram_tensor(
                "d_xnt", [128, NDC * N], BF16, kind="ExternalOutput").ap()
            self.d_kt = nc.dram_tensor(
                "d_kt", [128, NP * N], BF16, kind="ExternalOutput").ap()
            self.d_qt = nc.dram_tensor(
                "d_qt", [128, NP * N], BF16, kind="ExternalOutput").ap()
            self.d_va = nc.dram_tensor(
                "d_va", [128, NT * HL * (DH + 1)], BF16, kind="ExternalOutput").ap()
            self.d_at = nc.dram_tensor(
                "d_at", [128, NP * N], BF16, kind="ExternalOutput").ap()

    def build(self):
        nc = self.nc
        with ExitStack() as stack:
            tc = stack.enter_context(tile.TileContext(nc))
            self.tc = tc
            ep = stack.enter_context

            consts = ep(tc.tile_pool(name="consts", bufs=1))
            xnt_pool = ep(tc.tile_pool(name="xnt_p", bufs=1))
            qt_pool = ep(tc.tile_pool(name="qt_p", bufs=NP))
            kt_pool = ep(tc.tile_pool(name="kt_p", bufs=NP))
            vaug_pool = ep(tc.tile_pool(name="vaug_p", bufs=1))
            attnt_pool = ep(tc.tile_pool(name="attnt_p", bufs=NP))
            wv_pool = ep(tc.tile_pool(name="wv_p", bufs=NDC))
            wo_pool = ep(tc.tile_pool(name="wo_p", bufs=NP))
            self.wqk_pool = ep(tc.tile_pool(name="wqk_p", bufs=2 * NP * NDC))
            self.xload = ep(tc.tile_pool(name="xload", bufs=4))
            self.xn_pool = ep(tc.tile_pool(name="xn_p", bufs=3))
            self.stats = ep(tc.tile_pool(name="stats", bufs=8))
            self.expp = ep(tc.tile_pool(name="expp", bufs=9))
            self.svp = ep(tc.tile_pool(name="svp", bufs=4))
            self.lbp = ep(tc.tile_pool(name="lbp", bufs=2))
            self.obp = ep(tc.tile_pool(name="obp", bufs=3))

            self.ident = consts.tile([128, 128], BF16)
            make_identity(nc, self.ident)
            self.eps_t = consts.tile([128, 1], F32)
            nc.vector.memset(self.eps_t, EPS)

            self.xnt = xnt_pool.tile([128, NDC, N], BF16, tag="xnt", name="xnt")
            self.qt = [
                qt_pool.tile([128, N], BF16, tag="qt", name="qt") for _ in range(NP)
            ]
            self.kt = [
                kt_pool.tile([128, N], BF16, tag="kt", name="kt") for _ in range(NP)
            ]
            self.vaug = vaug_pool.tile(
                [128, NT, HL, DH + 1], BF16, tag="vaug", name="vaug"
            )
            self.attnt = [
                attnt_pool.tile([128, N], BF16, tag="attnt", name="attnt")
                for _ in range(NP)
            ]

            # ones column of vaug (softmax denominator trick)
            ones_t = consts.tile([128, NT * HL], BF16, tag="ones", name="ones")
            nc.vector.memset(ones_t, 1.0)
            nc.vector.tensor_copy(
                out=self.vaug[:, :, :, DH : DH + 1],
                in_=ones_t.rearrange("p (m h) -> p m h", m=NT)[:, :, :, None],
            )

            # v weights (needed from the first m-tile's v-projection)
            self.wv_sb = [
                wv_pool.tile([128, EL], BF16, tag="wv", name="wv")
                for _ in range(NDC)
            ]
            self.wo_sb = [
                wo_pool.tile([128, D], BF16, tag="wo", name="wo") for _ in range(NP)
            ]

            # filler PSUM pool: 2 banks, shared by late m-tiles / projections /
            # out-proj interleaved into the attention phase
            self.ps_fill = ep(tc.tile_pool(name="ps_fill", bufs=2, space="PSUM"))

            # ---------- prologue: first 4 m-tiles + pair-0 first chunks -------
            # minimal: just enough for group 0's kc=0-3; everything else is
            # produced inside group 0's kc loop right before its consumer
            with (
                tc.tile_pool(name="ps_pt", bufs=2, space="PSUM") as ps_pt,
                tc.tile_pool(name="ps_pv", bufs=2, space="PSUM") as ps_pv,
            ):
                for m in range(4):
                    self.m_unit(m, ps_pt, ps_pv, ln_on_act=True)
                self.proj_unit(self.kt[0], self.wqk_sb[("k", 0)], 0, ps_pv)
                self.proj_unit(self.qt[0], self.wqk_sb[("q", 0)], 0, ps_pv)

            # ---------- attention groups + interleaved filler -----------------
            # Emission order IS the dependency order for Tile: every unit must
            # be emitted before the first instruction that reads its output.
            # qt/kt filler units sit at late slots (12-14) INSIDE the group
            # preceding their consumer, because each group pre-issues the next
            # group's first score matmuls at its kc=15.
            proj_filler = {
                0: [("q", 0, 1)],
                1: [("q", 0, 2), ("k", 1, 0)],
                2: [("q", 0, 3), ("k", 1, 1), ("k", 1, 2)],
                3: [("k", 1, 3), ("q", 1, 0)],
                4: [("q", 1, 1), ("k", 2, 0)],
                5: [("q", 1, 2), ("k", 2, 1), ("k", 2, 2)],
                6: [("q", 1, 3), ("k", 2, 3), ("q", 2, 0)],
                7: [("q", 2, 1), ("k", 3, 0), ("k", 3, 1)],
                8: [("q", 2, 2), ("k", 3, 2), ("k", 3, 3)],
                9: [("q", 2, 3), ("q", 3, 0)],
                10: [("q", 3, 1)],
                11: [("q", 3, 2), ("q", 3, 3)],
            }
            filler_after = {
                15: [("op", 12), ("op", 13), ("op", 14), ("op", 15)],
            }

            # out-proj for finished q-blocks spreads inside the next group's
            # kc loop instead of bunching between groups (half-units so each
            # filler burst fits inside one exp window)
            def op_inline(ms):
                im = {}
                for i, m in enumerate(ms):
                    h0, h1 = self.op_unit_halves(m, self.ps_fill)
                    im[4 * i + 1] = [h0]
                    im[4 * i + 2] = [h1]
                return im
            # group 0 still consumes vaug[8..15] and kt[0] chunks 8-15 that the
            # prologue didn't produce: emit those units INSIDE its kc loop,
            # each before the iteration that first reads it.
            # group-0 inline production, emitted at the BOTTOM of iteration k
            # (after its scores/AV) so it never head-of-line blocks the PE.
            # Deadlines: kt chunk nc_j before scores(4j) -> slot <= 4j-2;
            # lnt(m) (LN+transpose -> xnt) before its kt chunk; v(m) before
            # AV(kc=m) -> slot <= m-1.
            def mk_lnt(m):
                return lambda: self.m_unit(
                    m, self.ps_ramp, self.ps_ramp, v_proj=False)

            def mk_v(m):
                return lambda: self.v_unit(m, self.ps_ramp)

            def mk_kt(ncb):
                return lambda: self.proj_unit(
                    self.kt[0], self.wqk_sb[("k", 0)], ncb, self.ps_fill)

            g0_inline = {
                0: [mk_lnt(4), mk_lnt(5)],
                1: [mk_lnt(6), mk_lnt(7), mk_v(4)],
                2: [mk_kt(1), mk_v(5)],
                3: [mk_lnt(8), mk_v(6)],
                4: [mk_lnt(9), mk_v(7)],
                5: [mk_lnt(10), mk_v(8)],
                6: [mk_lnt(11), mk_kt(2), mk_v(9)],
                7: [mk_lnt(12), mk_v(10)],
                8: [mk_lnt(13), mk_v(11)],
                9: [mk_lnt(14), mk_v(12)],
                10: [mk_lnt(15), mk_kt(3), mk_v(13)],
                11: [mk_v(14)],
                12: [mk_v(15)],
            }
            inline_map = {
                0: g0_inline,
                13: op_inline([0, 1, 2, 3]),
                14: op_inline([4, 5, 6, 7]),
                15: op_inline([8, 9, 10, 11]),
            }
            for g, items in proj_filler.items():
                im = inline_map.setdefault(g, {})
                for j, (dst, p_, ncb) in enumerate(items):
                    h0, h1 = self.proj_unit_halves(dst, p_, ncb)
                    im.setdefault(9 + 2 * j, []).append(h0)
                    im.setdefault(10 + 2 * j, []).append(h1)
            groups = [(p, qb) for p in range(NP) for qb in range(QB)]

            # group 0 runs with single-buffered scores (its exp cadence is
            # production-bound, not score-bound) freeing 2 PSUM banks for a
            # second ramp filler pool -> ~4 production units in flight
            with tc.tile_pool(name="ps_ot", bufs=2, space="PSUM") as ps_ot:
                with (
                    tc.tile_pool(name="ps_sc0", bufs=1, space="PSUM") as ps_sc0,
                    tc.tile_pool(name="ps_ramp", bufs=2, space="PSUM") as ps_ramp,
                ):
                    self.ps_ramp = ps_ramp
                    self.attn_group(
                        0, 0, ps_sc0, ps_ot, inline=inline_map.get(0),
                    )
                with tc.tile_pool(name="ps_sc", bufs=2, space="PSUM") as ps_sc:
                    pre = None
                    for gi, (p, qb) in enumerate(groups):
                        if gi == 0:
                            continue
                        if gi == 8:
                            # out-proj weights: late load, clear of the ramp's
                            # DMA window, well before the first op unit (g13)
                            for ec in range(NP):
                                nc.gpsimd.dma_start(
                                    out=self.wo_sb[ec],
                                    in_=self.wo[ec * 128 : (ec + 1) * 128, :],
                                )
                        nxt = groups[gi + 1] if gi + 1 < len(groups) else None
                        pre = self.attn_group(
                            qb, p, ps_sc, ps_ot,
                            inline=inline_map.get(gi),
                            first_sts=pre,
                            next_group=nxt,
                        )
                        for item in filler_after.get(gi, ()):
                            if item[0] == "op":
                                self.op_unit(item[1], self.ps_fill)

            if self.dump:
                nc.gpsimd.dma_start(
                    out=self.d_xnt, in_=self.xnt.rearrange("p a b -> p (a b)"))
                nc.gpsimd.dma_start(
                    out=self.d_va, in_=self.vaug.rearrange("p a b c -> p (a b c)"))
                for p in range(NP):
                    nc.gpsimd.dma_start(out=self.d_kt[:, p*N:(p+1)*N], in_=self.kt[p])
                    nc.gpsimd.dma_start(out=self.d_qt[:, p*N:(p+1)*N], in_=self.qt[p])
                    nc.gpsimd.dma_start(out=self.d_at[:, p*N:(p+1)*N], in_=self.attnt[p])

        nc.compile()
        return nc

    # -------------------------------------------------------------------- ops
    def dma_wqk(self, dst, p):
        nc = self.nc
        w = self.wq if dst == "q" else self.wk
        tiles = []
        for d in range(NDC):
            wt = self.wqk_pool.tile([128, 128], BF16, tag="w", name="w")
            nc.gpsimd.dma_start(
                out=wt, in_=w[d * 128 : (d + 1) * 128, p * 128 : (p + 1) * 128]
            )
            tiles.append(wt)
        self.wqk_sb[(dst, p)] = tiles

    def proj_unit(self, dst_tile, wts, ncb, psum_pool):
        """Project one 512-col n-chunk of q^T or k^T."""
        nc = self.nc
        ps = psum_pool.tile([128, 512], F32, tag="fill", name="pp")
        for d in range(NDC):
            nc.tensor.matmul(
                out=ps,
                lhsT=wts[d],
                rhs=self.xnt[:, d, ncb * 512 : (ncb + 1) * 512],
                start=(d == 0),
                stop=(d == NDC - 1),
            )
        nc.vector.tensor_copy(
            out=dst_tile[:, ncb * 512 : (ncb + 1) * 512], in_=ps
        )

    def proj_unit_halves(self, dst, p_, ncb):
        """proj_unit split into two ~1.1us thunks so each fits inside one
        exp window when used as inline filler.  Weight-tile lookup is
        deferred to emission time (the DMAs are issued mid-build)."""
        nc = self.nc
        state = {}

        def half(lo, hi):
            wts = self.wqk_sb[(dst, p_)]
            dst_tile = self.qt[p_] if dst == "q" else self.kt[p_]
            if lo == 0:
                state["ps"] = self.ps_fill.tile(
                    [128, 512], F32, tag="fill", name="pp")
            ps = state["ps"]
            for d in range(lo, hi):
                nc.tensor.matmul(
                    out=ps,
                    lhsT=wts[d],
                    rhs=self.xnt[:, d, ncb * 512 : (ncb + 1) * 512],
                    start=(d == 0),
                    stop=(d == NDC - 1),
                )
            if hi == NDC:
                nc.vector.tensor_copy(
                    out=dst_tile[:, ncb * 512 : (ncb + 1) * 512], in_=ps
                )

        return (lambda: half(0, NDC // 2)), (lambda: half(NDC // 2, NDC))

    def v_unit(self, m, ps_pv):
        """v natural projection for one m-tile (reads xnt, writes vaug)."""
        nc = self.nc
        pv = ps_pv.tile([128, EL], F32, tag="fill", name="pv")
        for d in range(NDC):
            nc.tensor.matmul(
                out=pv,
                lhsT=self.xnt[:, d, m * 128 : (m + 1) * 128],
                rhs=self.wv_sb[d],
                start=(d == 0),
                stop=(d == NDC - 1),
            )
        nc.vector.tensor_copy(
            out=self.vaug[:, m, :, 0:DH],
            in_=pv.rearrange("p (h d) -> p h d", h=HL),
        )

    def m_unit(self, m, ps_pt, ps_pv, ln_on_act=False, v_proj=True):
        """Load + LayerNorm + transpose (+ optional v-projection) for one
        128-token tile."""
        nc = self.nc
        xt = self.xload.tile([128, D], BF16, tag="xt", name="xt")
        nc.gpsimd.dma_start(out=xt, in_=self.x[m * 128 : (m + 1) * 128, :])
        if m == 0:
            # weight DMAs after the first x loads so x stays prioritized,
            # but before m0's v-projection reads wv_sb
            for d in range(NDC):
                nc.gpsimd.dma_start(
                    out=self.wv_sb[d], in_=self.wv[d * 128 : (d + 1) * 128, :]
                )
            self.dma_wqk("k", 0)
            self.dma_wqk("q", 0)
        st = self.stats.tile([128, 2, 6], F32, tag="bn", name="bn")
        nc.vector.bn_stats(out=st[:, 0, :], in_=xt[:, 0:512])
        nc.vector.bn_stats(out=st[:, 1, :], in_=xt[:, 512:1024])
        mv = self.stats.tile([128, 2], F32, tag="mv", name="mv")
        nc.vector.bn_aggr(out=mv, in_=st)
        sq = self.stats.tile([128, 1], F32, tag="sq", name="sq")
        nc.scalar.activation(
            out=sq,
            in_=mv[:, 1:2],
            func=mybir.ActivationFunctionType.Sqrt,
            bias=self.eps_t,
            scale=1.0,
        )
        rec = self.stats.tile([128, 1], F32, tag="rec", name="rec")
        nc.vector.reciprocal(out=rec, in_=sq)
        # -mu * rstd, for LN-apply as one ACT pass
        nmr = self.stats.tile([128, 1], F32, tag="nmr", name="nmr")
        nc.vector.tensor_scalar(
            out=nmr,
            in0=mv[:, 0:1],
            scalar1=rec,
            scalar2=-1.0,
            op0=mybir.AluOpType.mult,
            op1=mybir.AluOpType.mult,
        )
        # LN apply: ACT during the prologue (ACT idle, DVE is the prologue
        # critical path); DVE for tiles produced under the exp stream
        xn = self.xn_pool.tile([128, D], BF16, tag="xn", name="xn")
        if ln_on_act:
            nc.scalar.activation(
                out=xn,
                in_=xt,
                func=mybir.ActivationFunctionType.Identity,
                bias=nmr,
                scale=rec,
            )
        else:
            nc.vector.tensor_scalar(
                out=xn,
                in0=xt,
                scalar1=rec,
                scalar2=nmr,
                op0=mybir.AluOpType.mult,
                op1=mybir.AluOpType.add,
            )
        # all 8 transposes land in one single-bank bf16 PSUM tile -> one copy
        pt = ps_pt.tile([128, NDC, 128], BF16, tag="fill", name="pt")
        for d in range(NDC):
            nc.tensor.transpose(
                pt[:, d, :], xn[:, d * 128 : (d + 1) * 128], self.ident[:, :]
            )
        nc.vector.tensor_copy(
            out=self.xnt[:, :, m * 128 : (m + 1) * 128], in_=pt
        )
        if v_proj:
            self.v_unit(m, ps_pv)
        if m == 7:
            for p_ in range(1, NP):
                self.dma_wqk("k", p_)
                self.dma_wqk("q", p_)

    def score_unit(self, p, qb, kc, ps_sc):
        nc = self.nc
        qoff = qb * 512
        sts = ps_sc.tile([128, 1024], F32, tag="st", name="st")
        for hs in range(2):
            off = hs * 64
            nc.tensor.matmul(
                out=sts[:, hs * 512 : (hs + 1) * 512],
                lhsT=self.kt[p][off : off + 64, kc * 128 : (kc + 1) * 128],
                rhs=self.qt[p][off : off + 64, qoff : qoff + 512],
                start=True,
                stop=True,
            )
        return sts

    def attn_group(self, qb, p, ps_sc, ps_ot, inline=None, first_sts=None,
                   next_group=None):
        """One (head-pair, q-block) attention sweep.  `first_sts` is this
        group's kc=0 score tile when it was pre-issued by the previous group;
        `next_group` = (p', qb') gets its kc=0 scores issued before our last
        AV so the exp stream never stalls at the group boundary.  Returns the
        pre-issued tile for the next group."""
        nc = self.nc
        qoff = qb * 512

        ots = [
            ps_ot.tile([DH + 1, 512], F32, tag="ot", name="ot") for _ in range(2)
        ]
        sts = first_sts if first_sts is not None else self.score_unit(
            p, qb, 0, ps_sc)
        pre = None
        for kc in range(NT):
            if inline:
                for thunk in inline.get(kc, ()):
                    thunk()
            e = self.expp.tile([128, 1024], BF16, tag="exp", name="exp")
            nc.scalar.activation(
                out=e, in_=sts, func=mybir.ActivationFunctionType.Exp, scale=SCALE
            )
            # software pipeline: next chunk's scores issue before this AV so
            # the PE keeps streaming while ACT works on exp
            if kc + 1 < NT:
                sts = self.score_unit(p, qb, kc + 1, ps_sc)
            elif next_group is not None:
                pre = self.score_unit(next_group[0], next_group[1], 0, ps_sc)
            for hs in range(2):
                nc.tensor.matmul(
                    out=ots[hs],
                    lhsT=self.vaug[:, kc, 2 * p + hs, :],
                    rhs=e[:, hs * 512 : (hs + 1) * 512],
                    start=(kc == 0),
                    stop=(kc == NT - 1),
                )
        # epilogue: one quick copy releases each ot slot; the normalize
        # chain runs detached on DVE/GpSimd
        for hs in range(2):
            off = hs * 64
            sv = self.svp.tile([DH + 1, 512], F32, tag="sv", name="sv")
            nc.vector.tensor_copy(out=sv, in_=ots[hs])
            # custom-DVE recip needs a partition-0 operand: copy the
            # denominator row down first
            lraw = self.svp.tile([1, 512], F32, tag="lrow", name="lraw")
            nc.vector.tensor_copy(out=lraw, in_=ots[hs][DH : DH + 1, :])
            lrow = self.svp.tile([1, 512], F32, tag="lrow", name="lrow")
            nc.vector.reciprocal_approx_fast(out=lrow, in_=lraw)
            lb = self.lbp.tile([64, 512], F32, tag="lb", name="lb")
            nc.gpsimd.partition_broadcast(lb[:, :], lrow[:, :])
            nc.vector.tensor_mul(
                out=self.attnt[p][off : off + 64, qoff : qoff + 512],
                in0=sv[0:DH, :],
                in1=lb,
            )
        return pre

    def op_unit(self, m, ps_fill):
        """Output projection + store for one 128-token chunk."""
        h0, h1 = self.op_unit_halves(m, ps_fill)
        h0()
        h1()

    def op_unit_halves(self, m, ps_fill):
        """op_unit split per output half-column for inline filler use."""
        nc = self.nc
        state = {}

        def half(ns):
            if ns == 0:
                state["ob"] = self.obp.tile([128, D], F32, tag="ob", name="ob")
            ob = state["ob"]
            po = ps_fill.tile([128, 512], F32, tag="fill", name="po")
            for ec in range(NP):
                nc.tensor.matmul(
                    out=po,
                    lhsT=self.attnt[ec][:, m * 128 : (m + 1) * 128],
                    rhs=self.wo_sb[ec][:, ns * 512 : (ns + 1) * 512],
                    start=(ec == 0),
                    stop=(ec == NP - 1),
                )
            nc.vector.tensor_copy(out=ob[:, ns * 512 : (ns + 1) * 512], in_=po)
            if ns == 1:
                nc.gpsimd.dma_start(
                    out=self.out[m * 128 : (m + 1) * 128, :], in_=ob
                )

        return (lambda: half(0)), (lambda: half(1))


def _build_nc():
    return _Kern().build()


def _get_nc():
    if "nc" not in _nc_cache:
        _nc_cache["nc"] = _build_nc()
    return _nc_cache["nc"]


def _make_in_maps(q, ln_gamma, ln_beta, W_q, W_kv, W_out):
    q = np.asarray(q, dtype=np.float32)
    g = np.asarray(ln_gamma, dtype=np.float32)
    beta = np.asarray(ln_beta, dtype=np.float32)
    W_q = np.asarray(W_q, dtype=np.float32)
    W_kv = np.asarray(W_kv, dtype=np.float32)
    W_out = np.asarray(W_out, dtype=np.float32)

    assert np.allclose(beta, 0.0, atol=1e-30), (
        "nonzero ln_beta not supported by this kernel build"
    )
    bf16 = ml_dtypes.bfloat16
    wq_full = (g[:, None] * W_q).astype(bf16)
    wk_full = (g[:, None] * W_kv[:, :E]).astype(bf16)
    wv_full = (g[:, None] * W_kv[:, E:]).astype(bf16)
    wo_full = W_out.astype(bf16)

    q_bf = q.astype(bf16)
    in_maps = []
    for c in range(NCORES):
        b, grp = c // 2, c % 2
        cols = slice(grp * EL, (grp + 1) * EL)
        in_maps.append(
            {
                "x": np.ascontiguousarray(q_bf[b]),
                "wq": np.ascontiguousarray(wq_full[:, cols]),
                "wk": np.ascontiguousarray(wk_full[:, cols]),
                "wv": np.ascontiguousarray(wv_full[:, cols]),
                "wo": np.ascontiguousarray(wo_full[cols, :]),
            }
        )
    return in_maps


def _gather(results):
    out = np.empty((B, N, D), dtype=np.float32)
    for b in range(B):
        out[b] = results[2 * b]["out"] + results[2 * b + 1]["out"]
    return out


def kernel(q, ln_gamma, ln_beta, W_q, W_kv, W_out):
    nc = _get_nc()
    in_maps = _make_in_maps(q, ln_gamma, ln_beta, W_q, W_kv, W_out)
    res = run_bass_kernel_spmd(nc, in_maps, core_ids=list(range(NCORES)))
    return _gather(res.results)


def kernel_traced(q, ln_gamma, ln_beta, W_q, W_kv, W_out):
    """Like kernel() but with NTFF profiling; returns (out, BassKernelResults)."""
    nc = _get_nc()
    in_maps = _make_in_maps(q, ln_gamma, ln_beta, W_q, W_kv, W_out)
    res = run_bass_kernel_spmd(nc, in_maps, core_ids=list(range(NCORES)), trace=True)
    return _gather(res.results), res


# revision 68
# speedup vs baseline: 1.0180x; 1.0180x over previous
"""Fused LayerNorm + 16-head self-attention + output projection on 8 NeuronCores.

Sharding: core c = (batch b = c//2, head-group g = c%2).  Data parallel over
the 4 batches; tensor parallel over head groups (8 heads each, Megatron-style
column split of W_q/W_kv and row split of W_out).  The two partial outputs
per batch are summed on the host.

v3 design notes (single-pass pipelined schedule, all-bf16 matmuls):
  The scalar engine's softmax exp stream (256 x [128,1024] ~= 285us) and the
  PE matmul stream (~420us of slices) are co-bottlenecks; the schedule keeps
  both streaming with no stage barriers.
  - q is processed in four 512-wide blocks (qb); heads in four pairs (p).
    Per (p,qb,kc): two score matmuls land in one [128,1024] PSUM tile
    (the two heads run concurrently on disjoint PE row groups 0-63/64-127),
    ONE exp covers both heads, two AV matmuls accumulate per-head
    [65,512] outputs (ones column in V gives the softmax denominator).
    The kc loop is software-pipelined: scores(kc+1) issue before AV(kc)
    so the PE never head-of-line blocks on the exp it feeds.
  - Stage-B PSUM = scores 2x2 banks + AV 2x1 = 6 banks; a 2-bank filler
    pool serves everything else (late LN/transpose/v tiles, remaining
    q/k projections, out-proj) interleaved between attention groups as
    PE gap-filler under the ACT exp stream.
  - LN apply runs on GpSimd (tensor_scalar with per-partition -mu/rstd),
    keeping ACT exp-only during steady state.
"""

from contextlib import ExitStack

import numpy as np
import ml_dtypes

import concourse.bacc as bacc
import concourse.tile as tile
from concourse import mybir
from concourse.bass_utils import run_bass_kernel_spmd
from concourse.masks import make_identity

F32 = mybir.dt.float32
BF16 = mybir.dt.bfloat16

B, N, D = 4, 2048, 1024
H_TOT, DH, E = 16, 64, 1024
NCORES = 8
HL = 8            # heads per core
EL = HL * DH      # 512 local embed
NT = N // 128     # 16 token tiles
NDC = D // 128    # 8 contraction chunks
NP = 4            # head pairs per core
QB = 4            # q blocks of 512
SCALE = float(DH) ** -0.5
EPS = 1e-5

_nc_cache = {}


class _Kern:
    def __init__(self, dump=False):
        self.dump = dump
        self.nc = bacc.Bacc("TRN2", target_bir_lowering=False)
        nc = self.nc
        self.x = nc.dram_tensor("x", [N, D], BF16, kind="ExternalInput").ap()
        self.wq = nc.dram_tensor("wq", [D, EL], BF16, kind="ExternalInput").ap()
        self.wk = nc.dram_tensor("wk", [D, EL], BF16, kind="ExternalInput").ap()
        self.wv = nc.dram_tensor("wv", [D, EL], BF16, kind="ExternalInput").ap()
        self.wo = nc.dram_tensor("wo", [EL, D], BF16, kind="ExternalInput").ap()
        self.out = nc.dram_tensor("out", [N, D], F32, kind="ExternalOutput").ap()
        self.wqk_sb = {}
        if dump:
            self.d_xnt = nc.dram_tensor(
                "d_xnt", [128, NDC * N], BF16, kind="ExternalOutput").ap()
            self.d_kt = nc.dram_tensor(
                "d_kt", [128, NP * N], BF16, kind="ExternalOutput").ap()
            self.d_qt = nc.dram_tensor(
                "d_qt", [128, NP * N], BF16, kind="ExternalOutput").ap()
            self.d_va = nc.dram_tensor(
                "d_va", [128, NT * HL * (DH + 1)], BF16, kind="ExternalOutput").ap()
            self.d_at = nc.dram_tensor(
                "d_at", [128, NP * N], BF16, kind="ExternalOutput").ap()

    def build(self):
        nc = self.nc
        with ExitStack() as stack:
            tc = stack.enter_context(tile.TileContext(nc))
            self.tc = tc
            ep = stack.enter_context

            consts = ep(tc.tile_pool(name="consts", bufs=1))
            xnt_pool = ep(tc.tile_pool(name="xnt_p", bufs=1))
            qt_pool = ep(tc.tile_pool(name="qt_p", bufs=NP))
            kt_pool = ep(tc.tile_pool(name="kt_p", bufs=NP))
            vaug_pool = ep(tc.tile_pool(name="vaug_p", bufs=1))
            attnt_pool = ep(tc.tile_pool(name="attnt_p", bufs=NP))
            wv_pool = ep(tc.tile_pool(name="wv_p", bufs=NDC))
            wo_pool = ep(tc.tile_pool(name="wo_p", bufs=NP))
            self.wqk_pool = ep(tc.tile_pool(name="wqk_p", bufs=2 * NP * NDC))
            self.xload = ep(tc.tile_pool(name="xload", bufs=4))
            self.xn_pool = ep(tc.tile_pool(name="xn_p", bufs=3))
            self.stats = ep(tc.tile_pool(name="stats", bufs=8))
            self.expp = ep(tc.tile_pool(name="expp", bufs=9))
            self.svp = ep(tc.tile_pool(name="svp", bufs=4))
            self.lbp = ep(tc.tile_pool(name="lbp", bufs=2))
            self.obp = ep(tc.tile_pool(name="obp", bufs=3))

            self.ident = consts.tile([128, 128], BF16)
            make_identity(nc, self.ident)
            self.eps_t = consts.tile([128, 1], F32)
            nc.vector.memset(self.eps_t, EPS)

            self.xnt = xnt_pool.tile([128, NDC, N], BF16, tag="xnt", name="xnt")
            self.qt = [
                qt_pool.tile([128, N], BF16, tag="qt", name="qt") for _ in range(NP)
            ]
            self.kt = [
                kt_pool.tile([128, N], BF16, tag="kt", name="kt") for _ in range(NP)
            ]
            self.vaug = vaug_pool.tile(
                [128, NT, HL, DH + 1], BF16, tag="vaug", name="vaug"
            )
            self.attnt = [
                attnt_pool.tile([128, N], BF16, tag="attnt", name="attnt")
                for _ in range(NP)
            ]

            # ones column of vaug (softmax denominator trick)
            ones_t = consts.tile([128, NT * HL], BF16, tag="ones", name="ones")
            nc.vector.memset(ones_t, 1.0)
            nc.vector.tensor_copy(
                out=self.vaug[:, :, :, DH : DH + 1],
                in_=ones_t.rearrange("p (m h) -> p m h", m=NT)[:, :, :, None],
            )

            # v weights (needed from the first m-tile's v-projection)
            self.wv_sb = [
                wv_pool.tile([128, EL], BF16, tag="wv", name="wv")
                for _ in range(NDC)
            ]
            self.wo_sb = [
                wo_pool.tile([128, D], BF16, tag="wo", name="wo") for _ in range(NP)
            ]

            # filler PSUM pool: 2 banks, shared by late m-tiles / projections /
            # out-proj interleaved into the attention phase
            self.ps_fill = ep(tc.tile_pool(name="ps_fill", bufs=2, space="PSUM"))

            # ---------- prologue: first 4 m-tiles + pair-0 first chunks -------
            # minimal: just enough for group 0's kc=0-3; everything else is
            # produced inside group 0's kc loop right before its consumer
            with (
                tc.tile_pool(name="ps_pt", bufs=2, space="PSUM") as ps_pt,
                tc.tile_pool(name="ps_pv", bufs=2, space="PSUM") as ps_pv,
            ):
                for m in range(4):
                    self.m_unit(m, ps_pt, ps_pv, ln_on_act=True)
                self.proj_unit(self.kt[0], self.wqk_sb[("k", 0)], 0, ps_pv)
                self.proj_unit(self.qt[0], self.wqk_sb[("q", 0)], 0, ps_pv)

            # ---------- attention groups + interleaved filler -----------------
            # Emission order IS the dependency order for Tile: every unit must
            # be emitted before the first instruction that reads its output.
            # qt/kt filler units sit at late slots (12-14) INSIDE the group
            # preceding their consumer, because each group pre-issues the next
            # group's first score matmuls at its kc=15.
            proj_filler = {
                0: [("q", 0, 1)],
                1: [("q", 0, 2), ("k", 1, 0)],
                2: [("q", 0, 3), ("k", 1, 1), ("k", 1, 2)],
                3: [("k", 1, 3), ("q", 1, 0)],
                4: [("q", 1, 1), ("k", 2, 0)],
                5: [("q", 1, 2), ("k", 2, 1), ("k", 2, 2)],
                6: [("q", 1, 3), ("k", 2, 3), ("q", 2, 0)],
                7: [("q", 2, 1), ("k", 3, 0), ("k", 3, 1)],
                8: [("q", 2, 2), ("k", 3, 2), ("k", 3, 3)],
                9: [("q", 2, 3), ("q", 3, 0)],
                10: [("q", 3, 1)],
                11: [("q", 3, 2), ("q", 3, 3)],
            }
            filler_after = {
                15: [("op", 12), ("op", 13), ("op", 14), ("op", 15)],
            }

            # out-proj for finished q-blocks spreads inside the next group's
            # kc loop instead of bunching between groups (half-units so each
            # filler burst fits inside one exp window)
            def op_inline(ms):
                im = {}
                for i, m in enumerate(ms):
                    h0, h1 = self.op_unit_halves(m, self.ps_fill)
                    im[4 * i + 1] = [h0]
                    im[4 * i + 2] = [h1]
                return im
            # group 0 still consumes vaug[8..15] and kt[0] chunks 8-15 that the
            # prologue didn't produce: emit those units INSIDE its kc loop,
            # each before the iteration that first reads it.
            # group-0 inline production, emitted at the BOTTOM of iteration k
            # (after its scores/AV) so it never head-of-line blocks the PE.
            # Deadlines: kt chunk nc_j before scores(4j) -> slot <= 4j-2;
            # lnt(m) (LN+transpose -> xnt) before its kt chunk; v(m) before
            # AV(kc=m) -> slot <= m-1.
            def mk_lnt(m):
                return lambda: self.m_unit(
                    m, self.ps_ramp, self.ps_ramp, v_proj=False)

            def mk_v(m):
                return lambda: self.v_unit(m, self.ps_ramp)

            def mk_kt(ncb):
                return lambda: self.proj_unit(
                    self.kt[0], self.wqk_sb[("k", 0)], ncb, self.ps_fill)

            g0_inline = {
                0: [mk_lnt(4), mk_lnt(5)],
                1: [mk_lnt(6), mk_lnt(7), mk_v(4)],
                2: [mk_kt(1), mk_v(5)],
                3: [mk_lnt(8), mk_v(6)],
                4: [mk_lnt(9), mk_v(7)],
                5: [mk_lnt(10), mk_v(8)],
                6: [mk_lnt(11), mk_kt(2), mk_v(9)],
                7: [mk_lnt(12), mk_v(10)],
                8: [mk_lnt(13), mk_v(11)],
                9: [mk_lnt(14), mk_v(12)],
                10: [mk_lnt(15), mk_kt(3), mk_v(13)],
                11: [mk_v(14)],
                12: [mk_v(15)],
            }
            inline_map = {
                0: g0_inline,
                13: op_inline([0, 1, 2, 3]),
                14: op_inline([4, 5, 6, 7]),
                15: op_inline([8, 9, 10, 11]),
            }
            for g, items in proj_filler.items():
                im = inline_map.setdefault(g, {})
                for j, (dst, p_, ncb) in enumerate(items):
                    h0, h1 = self.proj_unit_halves(dst, p_, ncb)
                    im.setdefault(9 + 2 * j, []).append(h0)
                    im.setdefault(10 + 2 * j, []).append(h1)
            groups = [(p, qb) for p in range(NP) for qb in range(QB)]

            # group 0 runs with single-buffered scores (its exp cadence is
            # production-bound, not score-bound) freeing 2 PSUM banks for a
            # second ramp filler pool -> ~4 production units in flight
            with tc.tile_pool(name="ps_ot", bufs=2, space="PSUM") as ps_ot:
                with (
                    tc.tile_pool(name="ps_sc0", bufs=1, space="PSUM") as ps_sc0,
                    tc.tile_pool(name="ps_ramp", bufs=2, space="PSUM") as ps_ramp,
                ):
                    self.ps_ramp = ps_ramp
                    self.attn_group(
                        0, 0, ps_sc0, ps_ot, inline=inline_map.get(0),
                    )
                with tc.tile_pool(name="ps_sc", bufs=2, space="PSUM") as ps_sc:
                    pre = None
                    for gi, (p, qb) in enumerate(groups):
                        if gi == 0:
                            continue
                        if gi == 8:
                            # out-proj weights: late load, clear of the ramp's
                            # DMA window, well before the first op unit (g13)
                            for ec in range(NP):
                                nc.gpsimd.dma_start(
                                    out=self.wo_sb[ec],
                                    in_=self.wo[ec * 128 : (ec + 1) * 128, :],
                                )
                        nxt = groups[gi + 1] if gi + 1 < len(groups) else None
                        pre = self.attn_group(
                            qb, p, ps_sc, ps_ot,
                            inline=inline_map.get(gi),
                            first_sts=pre,
                            next_group=nxt,
                        )
                        for item in filler_after.get(gi, ()):
                            if item[0] == "op":
                                self.op_unit(item[1], self.ps_fill)

            if self.dump:
                nc.gpsimd.dma_start(
                    out=self.d_xnt, in_=self.xnt.rearrange("p a b -> p (a b)"))
                nc.gpsimd.dma_start(
                    out=self.d_va, in_=self.vaug.rearrange("p a b c -> p (a b c)"))
                for p in range(NP):
                    nc.gpsimd.dma_start(out=self.d_kt[:, p*N:(p+1)*N], in_=self.kt[p])
                    nc.gpsimd.dma_start(out=self.d_qt[:, p*N:(p+1)*N], in_=self.qt[p])
                    nc.gpsimd.dma_start(out=self.d_at[:, p*N:(p+1)*N], in_=self.attnt[p])

        nc.compile()
        return nc

    # -------------------------------------------------------------------- ops
    def dma_wqk(self, dst, p):
        nc = self.nc
        w = self.wq if dst == "q" else self.wk
        tiles = []
        for d in range(NDC):
            wt = self.wqk_pool.tile([128, 128], BF16, tag="w", name="w")
            nc.gpsimd.dma_start(
                out=wt, in_=w[d * 128 : (d + 1) * 128, p * 128 : (p + 1) * 128]
            )
            tiles.append(wt)
        self.wqk_sb[(dst, p)] = tiles

    def proj_unit(self, dst_tile, wts, ncb, psum_pool):
        """Project one 512-col n-chunk of q^T or k^T."""
        nc = self.nc
        ps = psum_pool.tile([128, 512], F32, tag="fill", name="pp")
        for d in range(NDC):
            nc.tensor.matmul(
                out=ps,
                lhsT=wts[d],
                rhs=self.xnt[:, d, ncb * 512 : (ncb + 1) * 512],
                start=(d == 0),
                stop=(d == NDC - 1),
            )
        nc.vector.tensor_copy(
            out=dst_tile[:, ncb * 512 : (ncb + 1) * 512], in_=ps
        )

    def proj_unit_halves(self, dst, p_, ncb):
        """proj_unit split into two ~1.1us thunks so each fits inside one
        exp window when used as inline filler.  Weight-tile lookup is
        deferred to emission time (the DMAs are issued mid-build)."""
        nc = self.nc
        state = {}

        def half(lo, hi):
            wts = self.wqk_sb[(dst, p_)]
            dst_tile = self.qt[p_] if dst == "q" else self.kt[p_]
            if lo == 0:
                state["ps"] = self.ps_fill.tile(
                    [128, 512], F32, tag="fill", name="pp")
            ps = state["ps"]
            for d in range(lo, hi):
                nc.tensor.matmul(
                    out=ps,
                    lhsT=wts[d],
                    rhs=self.xnt[:, d, ncb * 512 : (ncb + 1) * 512],
                    start=(d == 0),
                    stop=(d == NDC - 1),
                )
            if hi == NDC:
                nc.vector.tensor_copy(
                    out=dst_tile[:, ncb * 512 : (ncb + 1) * 512], in_=ps
                )

        return (lambda: half(0, NDC // 2)), (lambda: half(NDC // 2, NDC))

    def v_unit(self, m, ps_pv):
        """v natural projection for one m-tile (reads xnt, writes vaug)."""
        nc = self.nc
        pv = ps_pv.tile([128, EL], F32, tag="fill", name="pv")
        for d in range(NDC):
            nc.tensor.matmul(
                out=pv,
                lhsT=self.xnt[:, d, m * 128 : (m + 1) * 128],
                rhs=self.wv_sb[d],
                start=(d == 0),
                stop=(d == NDC - 1),
            )
        nc.vector.tensor_copy(
            out=self.vaug[:, m, :, 0:DH],
            in_=pv.rearrange("p (h d) -> p h d", h=HL),
        )

    def m_unit(self, m, ps_pt, ps_pv, ln_on_act=False, v_proj=True):
        """Load + LayerNorm + transpose (+ optional v-projection) for one
        128-token tile."""
        nc = self.nc
        xt = self.xload.tile([128, D], BF16, tag="xt", name="xt")
        nc.gpsimd.dma_start(out=xt, in_=self.x[m * 128 : (m + 1) * 128, :])
        if m == 0:
            # weight DMAs after the first x loads so x stays prioritized,
            # but before m0's v-projection reads wv_sb
            for d in range(NDC):
                nc.gpsimd.dma_start(
                    out=self.wv_sb[d], in_=self.wv[d * 128 : (d + 1) * 128, :]
                )
            self.dma_wqk("k", 0)
            self.dma_wqk("q", 0)
        st = self.stats.tile([128, 2, 6], F32, tag="bn", name="bn")
        nc.vector.bn_stats(out=st[:, 0, :], in_=xt[:, 0:512])
        nc.vector.bn_stats(out=st[:, 1, :], in_=xt[:, 512:1024])
        mv = self.stats.tile([128, 2], F32, tag="mv", name="mv")
        nc.vector.bn_aggr(out=mv, in_=st)
        sq = self.stats.tile([128, 1], F32, tag="sq", name="sq")
        nc.scalar.activation(
            out=sq,
            in_=mv[:, 1:2],
            func=mybir.ActivationFunctionType.Sqrt,
            bias=self.eps_t,
            scale=1.0,
        )
        rec = self.stats.tile([128, 1], F32, tag="rec", name="rec")
        nc.vector.reciprocal(out=rec, in_=sq)
        # -mu * rstd, for LN-apply as one ACT pass
        nmr = self.stats.tile([128, 1], F32, tag="nmr", name="nmr")
        nc.vector.tensor_scalar(
            out=nmr,
            in0=mv[:, 0:1],
            scalar1=rec,
            scalar2=-1.0,
            op0=mybir.AluOpType.mult,
            op1=mybir.AluOpType.mult,
        )
        # LN apply: ACT during the prologue (ACT idle, DVE is the prologue
        # critical path); DVE for tiles produced under the exp stream
        xn = self.xn_pool.tile([128, D], BF16, tag="xn", name="xn")
        if ln_on_act:
            nc.scalar.activation(
                out=xn,
                in_=xt,
                func=mybir.ActivationFunctionType.Identity,
                bias=nmr,
                scale=rec,
            )
        else:
            nc.vector.tensor_scalar(
                out=xn,
                in0=xt,
                scalar1=rec,
                scalar2=nmr,
                op0=mybir.AluOpType.mult,
                op1=mybir.AluOpType.add,
            )
        # all 8 transposes land in one single-bank bf16 PSUM tile -> one copy
        pt = ps_pt.tile([128, NDC, 128], BF16, tag="fill", name="pt")
        for d in range(NDC):
            nc.tensor.transpose(
                pt[:, d, :], xn[:, d * 128 : (d + 1) * 128], self.ident[:, :]
            )
        nc.vector.tensor_copy(
            out=self.xnt[:, :, m * 128 : (m + 1) * 128], in_=pt
        )
        if v_proj:
            self.v_unit(m, ps_pv)
        if m == 7:
            for p_ in range(1, NP):
                self.dma_wqk("k", p_)
                self.dma_wqk("q", p_)

    def score_unit(self, p, qb, kc, ps_sc):
        nc = self.nc
        qoff = qb * 512
        sts = ps_sc.tile([128, 1024], F32, tag="st", name="st")
        for hs in range(2):
            off = hs * 64
            nc.tensor.matmul(
                out=sts[:, hs * 512 : (hs + 1) * 512],
                lhsT=self.kt[p][off : off + 64, kc * 128 : (kc + 1) * 128],
                rhs=self.qt[p][off : off + 64, qoff : qoff + 512],
                start=True,
                stop=True,
            )
        return sts

    def attn_group(self, qb, p, ps_sc, ps_ot, inline=None, first_sts=None,
                   next_group=None):
        """One (head-pair, q-block) attention sweep.  `first_sts` is this
        group's kc=0 score tile when it was pre-issued by the previous group;
        `next_group` = (p', qb') gets its kc=0 scores issued before our last
        AV so the exp stream never stalls at the group boundary.  Returns the
        pre-issued tile for the next group."""
        nc = self.nc
        qoff = qb * 512

        ots = [
            ps_ot.tile([DH + 1, 512], F32, tag="ot", name="ot") for _ in range(2)
        ]
        sts = first_sts if first_sts is not None else self.score_unit(
            p, qb, 0, ps_sc)
        pre = None
        for kc in range(NT):
            if inline:
                for thunk in inline.get(kc, ()):
                    thunk()
            e = self.expp.tile([128, 1024], BF16, tag="exp", name="exp")
            nc.scalar.activation(
                out=e, in_=sts, func=mybir.ActivationFunctionType.Exp, scale=SCALE
            )
            # software pipeline: next chunk's scores issue before this AV so
            # the PE keeps streaming while ACT works on exp
            if kc + 1 < NT:
                sts = self.score_unit(p, qb, kc + 1, ps_sc)
            elif next_group is not None:
                pre = self.score_unit(next_group[0], next_group[1], 0, ps_sc)
            for hs in range(2):
                nc.tensor.matmul(
                    out=ots[hs],
                    lhsT=self.vaug[:, kc, 2 * p + hs, :],
                    rhs=e[:, hs * 512 : (hs + 1) * 512],
                    start=(kc == 0),
                    stop=(kc == NT - 1),
                )
        # epilogue: one quick copy releases each ot slot; the normalize
        # chain runs detached on DVE/GpSimd
        for hs in range(2):
            off = hs * 64
            sv = self.svp.tile([DH + 1, 512], F32, tag="sv", name="sv")
            nc.vector.tensor_copy(out=sv, in_=ots[hs])
            # custom-DVE recip needs a partition-0 operand: copy the
            # denominator row down first
            lraw = self.svp.tile([1, 512], F32, tag="lrow", name="lraw")
            nc.vector.tensor_copy(out=lraw, in_=ots[hs][DH : DH + 1, :])
            lrow = self.svp.tile([1, 512], F32, tag="lrow", name="lrow")
            nc.vector.reciprocal_approx_fast(out=lrow, in_=lraw)
            lb = self.lbp.tile([64, 512], F32, tag="lb", name="lb")
            nc.gpsimd.partition_broadcast(lb[:, :], lrow[:, :])
            nc.vector.tensor_mul(
                out=self.attnt[p][off : off + 64, qoff : qoff + 512],
                in0=sv[0:DH, :],
                in1=lb,
            )
        return pre

    def op_unit(self, m, ps_fill):
        """Output projection + store for one 128-token chunk."""
        h0, h1 = self.op_unit_halves(m, ps_fill)
        h0()
        h1()

    def op_unit_halves(self, m, ps_fill):
        """op_unit split per output half-column for inline filler use."""
        nc = self.nc
        state = {}

        def half(ns):
            if ns == 0:
                state["ob"] = self.obp.tile([128, D], F32, tag="ob", name="ob")
            ob = state["ob"]
            po = ps_fill.tile([128, 512], F32, tag="fill", name="po")
            for ec in range(NP):
                nc.tensor.matmul(
                    out=po,
                    lhsT=self.attnt[ec][:, m * 128 : (m + 1) * 128],
                    rhs=self.wo_sb[ec][:, ns * 512 : (ns + 1) * 512],
                    start=(ec == 0),
                    stop=(ec == NP - 1),
                )
            nc.vector.tensor_copy(out=ob[:, ns * 512 : (ns + 1) * 512], in_=po)
            if ns == 1:
                nc.sync.dma_start(
                    out=self.out[m * 128 : (m + 1) * 128, :], in_=ob
                )

        return (lambda: half(0)), (lambda: half(1))


def _build_nc():
    return _Kern().build()


def _get_nc():
    if "nc" not in _nc_cache:
        _nc_cache["nc"] = _build_nc()
    return _nc_cache["nc"]


def _make_in_maps(q, ln_gamma, ln_beta, W_q, W_kv, W_out):
    q = np.asarray(q, dtype=np.float32)
    g = np.asarray(ln_gamma, dtype=np.float32)
    beta = np.asarray(ln_beta, dtype=np.float32)
    W_q = np.asarray(W_q, dtype=np.float32)
    W_kv = np.asarray(W_kv, dtype=np.float32)
    W_out = np.asarray(W_out, dtype=np.float32)

    assert np.allclose(beta, 0.0, atol=1e-30), (
        "nonzero ln_beta not supported by this kernel build"
    )
    bf16 = ml_dtypes.bfloat16
    wq_full = (g[:, None] * W_q).astype(bf16)
    wk_full = (g[:, None] * W_kv[:, :E]).astype(bf16)
    wv_full = (g[:, None] * W_kv[:, E:]).astype(bf16)
    wo_full = W_out.astype(bf16)

    q_bf = q.astype(bf16)
    in_maps = []
    for c in range(NCORES):
        b, grp = c // 2, c % 2
        cols = slice(grp * EL, (grp + 1) * EL)
        in_maps.append(
            {
                "x": np.ascontiguousarray(q_bf[b]),
                "wq": np.ascontiguousarray(wq_full[:, cols]),
                "wk": np.ascontiguousarray(wk_full[:, cols]),
                "wv": np.ascontiguousarray(wv_full[:, cols]),
                "wo": np.ascontiguousarray(wo_full[cols, :]),
            }
        )
    return in_maps


def _gather(results):
    out = np.empty((B, N, D), dtype=np.float32)
    for b in range(B):
        out[b] = results[2 * b]["out"] + results[2 * b + 1]["out"]
    return out


def kernel(q, ln_gamma, ln_beta, W_q, W_kv, W_out):
    nc = _get_nc()
    in_maps = _make_in_maps(q, ln_gamma, ln_beta, W_q, W_kv, W_out)
    res = run_bass_kernel_spmd(nc, in_maps, core_ids=list(range(NCORES)))
    return _gather(res.results)


def kernel_traced(q, ln_gamma, ln_beta, W_q, W_kv, W_out):
    """Like kernel() but with NTFF profiling; returns (out, BassKernelResults)."""
    nc = _get_nc()
    in_maps = _make_in_maps(q, ln_gamma, ln_beta, W_q, W_kv, W_out)
    res = run_bass_kernel_spmd(nc, in_maps, core_ids=list(range(NCORES)), trace=True)
    return _gather(res.results), res
